# revision 23
# baseline (speedup 1.0000x reference)
import numpy as np
from contextlib import ExitStack

E = 8
D = 128
H = 128
HP = H + E
HQ = HP + E
P = 128
GROUP = 16
EPS = 1e-5
HALF_LN_D = 0.5 * float(np.log(128.0))
N_CORES = 8

APPLY_ENGINES = "vvssgggg"
HLN_ENGINE = "v"

_NC_CACHE = {}


def _build_nc(b_loc, has_b1, has_ln1, has_b2, has_outgb, num_devices=1):
    import concourse.bass as bass
    import concourse.tile as tile
    from concourse import bacc, mybir, masks

    f32 = mybir.dt.float32
    bf16 = mybir.dt.bfloat16
    AO = mybir.AluOpType
    AF = mybir.ActivationFunctionType

    assert b_loc % (P * GROUP) == 0
    n_tiles = b_loc // P
    n_groups = b_loc // (P * GROUP)
    NPAIR = GROUP // 2

    nc = bacc.Bacc(
        "TRN2",
        target_bir_lowering=False,
        debug=False,
        enable_asserts=False,
        num_devices=num_devices,
    )

    xf_d = nc.dram_tensor("xf", [b_loc, E * D], bf16, kind="ExternalInput").ap()
    xt_d = nc.dram_tensor("xt", [n_tiles, D, E * P], bf16, kind="ExternalInput").ap()
    w1 = nc.dram_tensor("w1a", [D, E * HP], bf16, kind="ExternalInput").ap()
    w2 = nc.dram_tensor("w2bf", [H, E], bf16, kind="ExternalInput").ap()
    out = nc.dram_tensor("out", [b_loc, E * D], bf16, kind="ExternalOutput").ap()
    if has_b1:
        b1row = nc.dram_tensor("b1row", [1, H], bf16, kind="ExternalInput").ap()
    if has_ln1:
        g_ln1 = nc.dram_tensor("g_ln1", [P, H], bf16, kind="ExternalInput").ap()
        b_ln1 = nc.dram_tensor("b_ln1", [P, H], bf16, kind="ExternalInput").ap()
    if has_b2:
        eb2 = nc.dram_tensor("eb2", [P, E], f32, kind="ExternalInput").ap()
    if has_outgb:
        g_out = nc.dram_tensor("g_out", [P, D], bf16, kind="ExternalInput").ap()
        b_out = nc.dram_tensor("b_out", [P, D], bf16, kind="ExternalInput").ap()

    xf_p = xf_d.rearrange("(m two p) f -> m p two f", two=2, p=P)
    out_p = out.rearrange("(m two p) f -> m p two f", two=2, p=P)
    xt_p = xt_d.rearrange("(m two) d f -> m d two f", two=2)

    with tile.TileContext(nc) as tc, ExitStack() as ctx:
        _act_prev = [None]

        def act_ordered(inst):
            ins = inst.ins
            if _act_prev[0] is not None:
                tile.add_dep_helper(ins, _act_prev[0], sync=False,
                                    reason="act-table order")
            _act_prev[0] = ins
            return inst

        def act_load(set_id):
            return act_ordered(nc.scalar.add_instruction(
                mybir.InstLoadActFuncSet(
                    name=nc.get_next_instruction_name(), ins=[], outs=[],
                    act_func_set_id=set_id)))

        ENG = {"v": nc.vector, "g": nc.gpsimd}

        const_pool = ctx.enter_context(tc.tile_pool(name="const", bufs=1))
        ident_b = const_pool.tile([P, P], bf16)
        masks.make_identity(nc, ident_b[:])
        ones_d = const_pool.tile([D, 1], bf16)
        nc.vector.memset(ones_d[:], 1.0)
        w1_sb = const_pool.tile([D, E * HP], bf16)
        w1_3 = w1_sb.rearrange("d (e h) -> d e h", e=E)
        nc.sync.dma_start(w1_sb[:], w1)
        w2_sb = const_pool.tile([H, E], bf16)
        nc.sync.dma_start(w2_sb[:], w2)
        if has_b1:
            ones1 = const_pool.tile([1, P], bf16)
            nc.vector.memset(ones1[:], 1.0)
            b1_sb = const_pool.tile([1, H], bf16)
            nc.sync.dma_start(b1_sb[:], b1row)
        if has_ln1:
            gln_sb = const_pool.tile([P, H], bf16)
            nc.sync.dma_start(gln_sb[:], g_ln1)
            bln_sb = const_pool.tile([P, H], bf16)
            nc.sync.dma_start(bln_sb[:], b_ln1)
        if has_b2:
            eb2_sb = const_pool.tile([P, E], f32)
            nc.sync.dma_start(eb2_sb[:], eb2)
        if has_outgb:
            gout_sb = const_pool.tile([P, D], bf16)
            nc.sync.dma_start(gout_sb[:], g_out)
            bout_sb = const_pool.tile([P, D], bf16)
            nc.sync.dma_start(bout_sb[:], b_out)

        hld = const_pool.tile([P, 1], f32)
        nc.vector.memset(hld[:], HALF_LN_D)

        io_pool = ctx.enter_context(tc.tile_pool(name="io", bufs=2 * NPAIR + 1))
        xt_pool = ctx.enter_context(tc.tile_pool(name="xt", bufs=6))
        sq_pool = ctx.enter_context(tc.tile_pool(name="sq", bufs=4))
        hg_pool = ctx.enter_context(tc.tile_pool(name="hg", bufs=2 * NPAIR + 1))
        osb_pool = ctx.enter_context(tc.tile_pool(name="osb", bufs=6))
        sm_pool = ctx.enter_context(tc.tile_pool(name="sm", bufs=4))
        grp_pool = ctx.enter_context(tc.tile_pool(name="grp", bufs=2))
        ps_h = ctx.enter_context(tc.tile_pool(name="ps_h", bufs=3, space="PSUM"))
        ps_t = ctx.enter_context(tc.tile_pool(name="ps_t", bufs=2, space="PSUM"))
        ps_lg = ctx.enter_context(tc.tile_pool(name="ps_lg", bufs=2, space="PSUM"))

        def emit_phase1(g):
            st = {}
            mss = grp_pool.tile([P, GROUP * 2 * E], f32, tag="mss")
            st["mss4"] = mss4 = mss.rearrange("p (j k e) -> p j k e",
                                              j=GROUP, k=2)
            ln_mv = grp_pool.tile([P, GROUP * 2], f32, tag="ln_mv")
            st["ln3"] = ln3 = ln_mv.rearrange("p (j s) -> p j s", j=GROUP)
            zzr = grp_pool.tile([P, GROUP * E], f32, tag="zzr", name="zzr")
            st["zzr"] = zzr
            st["zzr3"] = zzr.rearrange("p (j e) -> p j e", j=GROUP)
            zz = grp_pool.tile([P, GROUP * E], f32, tag="zz", name="zz")
            st["zz"] = zz
            st["zz3"] = zz.rearrange("p (j e) -> p j e", j=GROUP)
            zs = grp_pool.tile([P, GROUP], f32, tag="zs", name="zs")
            st["zs"] = zs

            st["xfs"] = xfs = []
            st["hgs"] = hgs = []
            act_load(10)
            for jp in range(NPAIR):
                j0 = 2 * jp
                ip = g * NPAIR + jp
                xf = io_pool.tile([P, 2 * E * D], bf16, tag="xf", name=f"xf_{ip}")
                xf4 = xf.rearrange("p (two e d) -> p two e d", two=2, e=E)
                nc.sync.dma_start(xf4, xf_p[ip])
                xfs.append(xf)

                xt = xt_pool.tile([D, 2 * E * P], bf16, tag="xt", name=f"xt_{ip}")
                xt4 = xt.rearrange("d (two e b) -> d two e b", two=2, e=E)
                nc.sync.dma_start(xt4, xt_p[ip])

                sq = sq_pool.tile([D, 2 * E * P], bf16, tag="sq",
                                  name=f"sq_{ip}")
                sq4 = sq.rearrange("d (two e b) -> d two e b", two=2, e=E)
                nc.vector.tensor_mul(sq[:], xt[:], xt[:])

                ph = ps_h.tile([P, 2 * HQ], f32, tag="ph", name=f"ph_{ip}")
                ph3 = ph.rearrange("p (two q) -> p two q", two=2)
                for jj in range(2):
                    for e in range(E):
                        nc.tensor.matmul(
                            ph3[:, jj, 0:HP], xt4[:, jj, e], w1_3[:, e],
                            start=(e == 0), stop=(e == E - 1 and not has_b1),
                        )
                    if has_b1:
                        nc.tensor.matmul(ph3[:, jj, 0:H], ones1[:],
                                         b1_sb[:], start=False, stop=True)
                    for e in range(E):
                        nc.tensor.matmul(
                            ph3[:, jj, HP + e:HP + e + 1], sq4[:, jj, e],
                            ones_d[:], start=True, stop=True,
                        )

                hg = hg_pool.tile([P, 2 * H], bf16, tag="hg", name=f"hg_{ip}")
                act_ordered(nc.scalar.activation(
                    hg.rearrange("p (two h) -> p two h", two=2),
                    ph3[:, :, 0:H], AF.Gelu, bias=0.0, scale=1.0))
                hgs.append(hg)

                nc.vector.tensor_copy(mss4[:, j0:j0 + 2, :, :],
                                      ph3[:, :, H:HQ].rearrange(
                                          "p two (k e) -> p two k e", k=2))

                for jj in range(2):
                    s1 = sm_pool.tile([P, 6], f32, tag="s1",
                                      name=f"s1_{g}_{j0 + jj}")
                    nc.vector.bn_stats(s1[:], hg[:, jj * H:(jj + 1) * H])
                    nc.vector.bn_aggr(ln3[:, j0 + jj], s1[:])
            return st

        def emit_rest(g, st):
            mss4 = st["mss4"]
            ln3 = st["ln3"]
            zzr = st["zzr"]
            zzr3 = st["zzr3"]
            zz = st["zz"]
            zz3 = st["zz3"]
            zs = st["zs"]
            xfs = st["xfs"]
            hgs = st["hgs"]

            act_load(6)
            veps = grp_pool.tile([P, GROUP], f32, tag="veps")
            nc.vector.tensor_scalar(veps[:], ln3[:, :, 1], EPS, None, AO.add)
            lnv = grp_pool.tile([P, GROUP], f32, tag="lnv")
            act_ordered(nc.scalar.activation(lnv[:], veps[:], AF.Ln,
                                             bias=0.0, scale=1.0))
            rs1 = grp_pool.tile([P, GROUP], f32, tag="rs1")
            act_ordered(nc.scalar.activation(rs1[:], lnv[:], AF.Exp,
                                             bias=0.0, scale=-0.5))
            mb1n = grp_pool.tile([P, GROUP], f32, tag="mb1n")
            nc.vector.scalar_tensor_tensor(mb1n[:], ln3[:, :, 0], -1.0, rs1[:],
                                           AO.mult, AO.mult)

            for jp in range(NPAIR):
                j0 = 2 * jp
                pt = ps_t.tile([P, 2 * P], bf16, tag="pt", name=f"pt_{g}_{jp}")
                for jj in range(2):
                    j = j0 + jj
                    hln = sm_pool.tile([P, H], bf16, tag="hln",
                                       name=f"hln_{g}_{j}")
                    hgj = hgs[jp][:, jj * H:(jj + 1) * H]
                    if HLN_ENGINE == "s":
                        act_ordered(nc.scalar.activation(
                            hln[:], hgj, AF.Identity,
                            bias=mb1n[:, j:j + 1], scale=rs1[:, j:j + 1]))
                    else:
                        ENG[HLN_ENGINE].tensor_scalar(
                            hln[:], hgj, rs1[:, j:j + 1], mb1n[:, j:j + 1],
                            AO.mult, AO.add)
                    if has_ln1:
                        nc.vector.tensor_mul(hln[:], hln[:], gln_sb[:])
                        nc.vector.tensor_add(hln[:], hln[:], bln_sb[:])
                    nc.tensor.matmul(pt[:, jj * P:(jj + 1) * P], hln[:],
                                     ident_b[:], is_transpose=True)
                hlt = sm_pool.tile([P, 2 * P], bf16, tag="hlt",
                                   name=f"hlt_{g}_{jp}")
                act_ordered(nc.scalar.activation(hlt[:], pt[:], AF.Copy))
                plg = ps_lg.tile([P, 2 * E], f32, tag="plg", name=f"plg_{g}_{jp}")
                for jj in range(2):
                    nc.tensor.matmul(plg[:, jj * E:(jj + 1) * E],
                                     hlt[:, jj * P:(jj + 1) * P], w2_sb[:],
                                     start=True, stop=True)
                act_ordered(nc.scalar.activation(zzr3[:, j0:j0 + 2], plg[:],
                                                 AF.Copy))

            zzr3 = st["zzr3"]
            act_ordered(nc.scalar.activation(zz[:], zzr[:], AF.Exp,
                                             bias=0.0, scale=1.0))
            if has_b2:
                for j in range(GROUP):
                    nc.vector.tensor_mul(zz3[:, j], zz3[:, j], eb2_sb[:])
            nc.vector.reduce_sum(zs[:], zz3[:], axis=mybir.AxisListType.X)

            msx = mss4[:, :, 0, :]
            sqx = mss4[:, :, 1, :]
            msq = grp_pool.tile([P, GROUP * E], f32, tag="msq")
            nc.vector.scalar_tensor_tensor(msq[:], msx, 1.0 / D, msx,
                                           AO.mult, AO.mult)
            M2x = grp_pool.tile([P, GROUP * E], f32, tag="M2x")
            nc.gpsimd.tensor_sub(M2x[:], sqx, msq[:])

            ttz = grp_pool.tile([P, GROUP * E], f32, tag="ttz")
            nc.vector.tensor_mul(ttz[:], zz[:], zz[:])
            uu = grp_pool.tile([P, GROUP * E], f32, tag="uu")
            uu3 = uu.rearrange("p (j e) -> p j e", j=GROUP)
            nc.vector.tensor_mul(uu[:], ttz[:], M2x[:])
            zeps = grp_pool.tile([P, GROUP], f32, tag="zeps")
            nc.vector.scalar_tensor_tensor(zeps[:], zs[:], float(D) * EPS, zs[:],
                                           AO.mult, AO.mult)
            u2 = grp_pool.tile([P, GROUP * E], f32, tag="u2")
            u23 = u2.rearrange("p (j e) -> p j e", j=GROUP)
            for j in range(GROUP):
                nc.vector.tensor_scalar(u23[:, j], uu3[:, j], zeps[:, j:j + 1],
                                        None, AO.add)
            l2 = grp_pool.tile([P, GROUP * E], f32, tag="l2")
            act_ordered(nc.scalar.activation(l2[:], u2[:], AF.Ln,
                                             bias=0.0, scale=1.0))
            qq = grp_pool.tile([P, GROUP * E], f32, tag="qq")
            act_ordered(nc.scalar.activation(qq[:], l2[:], AF.Exp,
                                             bias=hld[:], scale=-0.5))
            aa = grp_pool.tile([P, GROUP * E], f32, tag="aa")
            aa3 = aa.rearrange("p (j e) -> p j e", j=GROUP)
            nc.vector.tensor_mul(aa[:], zz[:], qq[:])
            bbn = grp_pool.tile([P, GROUP * E], f32, tag="bbn")
            bbn3 = bbn.rearrange("p (j e) -> p j e", j=GROUP)
            nc.vector.scalar_tensor_tensor(bbn[:], msx, -1.0 / D, aa[:],
                                           AO.mult, AO.mult)

            for jp in range(NPAIR):
                j0 = 2 * jp
                ip = g * NPAIR + jp
                xf = xfs[jp]
                osb = osb_pool.tile([P, 2 * E * D], bf16, tag="osb",
                                    name=f"osb_{ip}")
                for jj in range(2):
                    j = j0 + jj
                    for e in range(E):
                        lo = (jj * E + e) * D
                        sl_o = osb[:, lo:lo + D]
                        sl_x = xf[:, lo:lo + D]
                        ename = APPLY_ENGINES[e]
                        if ename == "s":
                            act_ordered(nc.scalar.activation(
                                sl_o, sl_x, AF.Identity,
                                bias=bbn3[:, j, e:e + 1],
                                scale=aa3[:, j, e:e + 1]))
                        else:
                            ENG[ename].tensor_scalar(
                                sl_o, sl_x,
                                aa3[:, j, e:e + 1], bbn3[:, j, e:e + 1],
                                AO.mult, AO.add,
                            )
                        if has_outgb:
                            nc.vector.tensor_mul(sl_o, sl_o, gout_sb[:])
                            nc.vector.tensor_add(sl_o, sl_o, bout_sb[:])
                nc.scalar.dma_start(
                    out_p[ip], osb.rearrange("p (two e d) -> p two e d",
                                             two=2, e=E))

        prev = None
        for g in range(n_groups):
            st = emit_phase1(g)
            if prev is not None:
                emit_rest(g - 1, prev)
            prev = st
        emit_rest(n_groups - 1, prev)

    nc.compile()
    return nc


def _get_nc(b_loc, flags, num_devices):
    key = (b_loc, flags, num_devices)
    if key not in _NC_CACHE:
        _NC_CACHE[key] = _build_nc(b_loc, *flags, num_devices=num_devices)
    return _NC_CACHE[key]


def kernel(**inputs):
    import ml_dtypes
    from concourse.bass_utils import run_bass_kernel_spmd

    features = np.asarray(inputs["features"], dtype=np.float32)
    gate_w1 = np.asarray(inputs["gate_w1"], dtype=np.float32)
    gate_b1 = np.asarray(inputs["gate_b1"], dtype=np.float32)
    ln1_g = np.asarray(inputs["ln1_g"], dtype=np.float32)
    ln1_b = np.asarray(inputs["ln1_b"], dtype=np.float32)
    gate_w2 = np.asarray(inputs["gate_w2"], dtype=np.float32)
    gate_b2 = np.asarray(inputs["gate_b2"], dtype=np.float32)
    out_g = np.asarray(inputs["out_g"], dtype=np.float32)
    out_b = np.asarray(inputs["out_b"], dtype=np.float32)

    e, B, d = features.shape
    assert e == E and d == D
    assert B % (N_CORES * P * GROUP) == 0
    b_loc = B // N_CORES
    n_tiles = b_loc // P

    has_b1 = bool(np.any(gate_b1 != 0))
    has_ln1 = bool(np.any(ln1_g != 1) or np.any(ln1_b != 0))
    has_b2 = bool(np.any(gate_b2 != 0))
    has_outgb = bool(np.any(out_g != 1) or np.any(out_b != 0))
    flags = (has_b1, has_ln1, has_b2, has_outgb)

    nc = _get_nc(b_loc, flags, num_devices=1)

    bf = ml_dtypes.bfloat16
    w1aug = np.zeros((D, E, HP), dtype=np.float32)
    w1aug[:, :, 0:H] = gate_w1.reshape(E, D, H).transpose(1, 0, 2)
    for ei in range(E):
        w1aug[:, ei, H + ei] = 1.0
    common = {
        "w1a": np.ascontiguousarray(w1aug.reshape(D, E * HP)).astype(bf),
        "w2bf": np.ascontiguousarray(gate_w2.astype(bf)),
    }
    if has_b1:
        common["b1row"] = np.ascontiguousarray(gate_b1.reshape(1, H).astype(bf))
    if has_ln1:
        common["g_ln1"] = np.ascontiguousarray(np.tile(ln1_g, (P, 1)).astype(bf))
        common["b_ln1"] = np.ascontiguousarray(np.tile(ln1_b, (P, 1)).astype(bf))
    if has_b2:
        common["eb2"] = np.ascontiguousarray(
            np.tile(np.exp(gate_b2.astype(np.float64)).astype(np.float32), (P, 1)))
    if has_outgb:
        common["g_out"] = np.ascontiguousarray(np.tile(out_g, (P, 1)).astype(bf))
        common["b_out"] = np.ascontiguousarray(np.tile(out_b, (P, 1)).astype(bf))

    fco = features.reshape(E, N_CORES, n_tiles, P, D)
    xf_all = fco.transpose(1, 2, 3, 0, 4).astype(bf)
    xt_all = fco.transpose(1, 2, 4, 0, 3).astype(bf)

    in_maps = []
    for c in range(N_CORES):
        m = dict(common)
        m["xf"] = np.ascontiguousarray(xf_all[c].reshape(b_loc, E * D))
        m["xt"] = np.ascontiguousarray(xt_all[c].reshape(n_tiles, D, E * P))
        in_maps.append(m)

    res = run_bass_kernel_spmd(nc, in_maps, core_ids=list(range(N_CORES)))
    global LAST_RESULTS
    LAST_RESULTS = res
    out = np.empty((E, B, D), dtype=np.float32)
    for c in range(N_CORES):
        oc = res.results[c]["out"].reshape(b_loc, E, D)
        out[:, c * b_loc:(c + 1) * b_loc, :] = oc.transpose(1, 0, 2)
    return out


LAST_RESULTS = None


# revision 24
# speedup vs baseline: 1.0231x; 1.0231x over previous
import numpy as np
from contextlib import ExitStack

E = 8
D = 128
H = 128
HP = H + E
HQ = HP + E
P = 128
GROUP = 16
EPS = 1e-5
HALF_LN_D = 0.5 * float(np.log(128.0))
N_CORES = 8

APPLY_ENGINES = "vvssgggg"
HLN_ENGINE = "v"

_NC_CACHE = {}


def _build_nc(b_loc, has_b1, has_ln1, has_b2, has_outgb, num_devices=1):
    import concourse.bass as bass
    import concourse.tile as tile
    from concourse import bacc, mybir, masks

    f32 = mybir.dt.float32
    bf16 = mybir.dt.bfloat16
    AO = mybir.AluOpType
    AF = mybir.ActivationFunctionType

    assert b_loc % (P * GROUP) == 0
    n_tiles = b_loc // P
    n_groups = b_loc // (P * GROUP)
    NPAIR = GROUP // 2

    nc = bacc.Bacc(
        "TRN2",
        target_bir_lowering=False,
        debug=False,
        enable_asserts=False,
        num_devices=num_devices,
    )

    xf_d = nc.dram_tensor("xf", [b_loc, E * D], bf16, kind="ExternalInput").ap()
    xt_d = nc.dram_tensor("xt", [n_tiles, D, E * P], bf16, kind="ExternalInput").ap()
    w1 = nc.dram_tensor("w1a", [D, E * HP], bf16, kind="ExternalInput").ap()
    w2 = nc.dram_tensor("w2bf", [H, E], bf16, kind="ExternalInput").ap()
    out = nc.dram_tensor("out", [b_loc, E * D], bf16, kind="ExternalOutput").ap()
    if has_b1:
        b1row = nc.dram_tensor("b1row", [1, H], bf16, kind="ExternalInput").ap()
    if has_ln1:
        g_ln1 = nc.dram_tensor("g_ln1", [P, H], bf16, kind="ExternalInput").ap()
        b_ln1 = nc.dram_tensor("b_ln1", [P, H], bf16, kind="ExternalInput").ap()
    if has_b2:
        eb2 = nc.dram_tensor("eb2", [P, E], f32, kind="ExternalInput").ap()
    if has_outgb:
        g_out = nc.dram_tensor("g_out", [P, D], bf16, kind="ExternalInput").ap()
        b_out = nc.dram_tensor("b_out", [P, D], bf16, kind="ExternalInput").ap()

    xf_p = xf_d.rearrange("(m two p) f -> m p two f", two=2, p=P)
    out_p = out.rearrange("(m two p) f -> m p two f", two=2, p=P)
    xt_p = xt_d.rearrange("(m two) d f -> m d two f", two=2)

    with tile.TileContext(nc) as tc, ExitStack() as ctx:
        _act_prev = [None]

        def act_ordered(inst):
            ins = inst.ins
            if _act_prev[0] is not None:
                tile.add_dep_helper(ins, _act_prev[0], sync=False,
                                    reason="act-table order")
            _act_prev[0] = ins
            return inst

        def act_load(set_id):
            return act_ordered(nc.scalar.add_instruction(
                mybir.InstLoadActFuncSet(
                    name=nc.get_next_instruction_name(), ins=[], outs=[],
                    act_func_set_id=set_id)))

        ENG = {"v": nc.vector, "g": nc.gpsimd}

        const_pool = ctx.enter_context(tc.tile_pool(name="const", bufs=1))
        ident_b = const_pool.tile([P, P], bf16)
        masks.make_identity(nc, ident_b[:])
        ones_d = const_pool.tile([D, 1], bf16)
        nc.vector.memset(ones_d[:], 1.0)
        w1_sb = const_pool.tile([D, E * HP], bf16)
        w1_3 = w1_sb.rearrange("d (e h) -> d e h", e=E)
        nc.sync.dma_start(w1_sb[:], w1)
        w2_sb = const_pool.tile([H, E], bf16)
        nc.sync.dma_start(w2_sb[:], w2)
        if has_b1:
            ones1 = const_pool.tile([1, P], bf16)
            nc.vector.memset(ones1[:], 1.0)
            b1_sb = const_pool.tile([1, H], bf16)
            nc.sync.dma_start(b1_sb[:], b1row)
        if has_ln1:
            gln_sb = const_pool.tile([P, H], bf16)
            nc.sync.dma_start(gln_sb[:], g_ln1)
            bln_sb = const_pool.tile([P, H], bf16)
            nc.sync.dma_start(bln_sb[:], b_ln1)
        if has_b2:
            eb2_sb = const_pool.tile([P, E], f32)
            nc.sync.dma_start(eb2_sb[:], eb2)
        if has_outgb:
            gout_sb = const_pool.tile([P, D], bf16)
            nc.sync.dma_start(gout_sb[:], g_out)
            bout_sb = const_pool.tile([P, D], bf16)
            nc.sync.dma_start(bout_sb[:], b_out)

        hld = const_pool.tile([P, 1], f32)
        nc.vector.memset(hld[:], HALF_LN_D)

        io_pool = ctx.enter_context(tc.tile_pool(name="io", bufs=2 * NPAIR + 1))
        xt_pool = ctx.enter_context(tc.tile_pool(name="xt", bufs=6))
        sq_pool = ctx.enter_context(tc.tile_pool(name="sq", bufs=4))
        hg_pool = ctx.enter_context(tc.tile_pool(name="hg", bufs=2 * NPAIR + 1))
        osb_pool = ctx.enter_context(tc.tile_pool(name="osb", bufs=6))
        sm_pool = ctx.enter_context(tc.tile_pool(name="sm", bufs=4))
        grp_pool = ctx.enter_context(tc.tile_pool(name="grp", bufs=2))
        ps_h = ctx.enter_context(tc.tile_pool(name="ps_h", bufs=4, space="PSUM"))
        ps_t = ctx.enter_context(tc.tile_pool(name="ps_t", bufs=2, space="PSUM"))
        ps_lg = ctx.enter_context(tc.tile_pool(name="ps_lg", bufs=2, space="PSUM"))

        def emit_phase1(g):
            st = {}
            mss = grp_pool.tile([P, GROUP * 2 * E], f32, tag="mss")
            st["mss4"] = mss4 = mss.rearrange("p (j k e) -> p j k e",
                                              j=GROUP, k=2)
            ln_mv = grp_pool.tile([P, GROUP * 2], f32, tag="ln_mv")
            st["ln3"] = ln3 = ln_mv.rearrange("p (j s) -> p j s", j=GROUP)
            zzr = grp_pool.tile([P, GROUP * E], f32, tag="zzr", name="zzr")
            st["zzr"] = zzr
            st["zzr3"] = zzr.rearrange("p (j e) -> p j e", j=GROUP)
            zz = grp_pool.tile([P, GROUP * E], f32, tag="zz", name="zz")
            st["zz"] = zz
            st["zz3"] = zz.rearrange("p (j e) -> p j e", j=GROUP)
            zs = grp_pool.tile([P, GROUP], f32, tag="zs", name="zs")
            st["zs"] = zs

            st["xfs"] = xfs = []
            st["hgs"] = hgs = []
            act_load(10)
            for jp in range(NPAIR):
                j0 = 2 * jp
                ip = g * NPAIR + jp
                xf = io_pool.tile([P, 2 * E * D], bf16, tag="xf", name=f"xf_{ip}")
                xf4 = xf.rearrange("p (two e d) -> p two e d", two=2, e=E)
                nc.sync.dma_start(xf4, xf_p[ip])
                xfs.append(xf)

                xt = xt_pool.tile([D, 2 * E * P], bf16, tag="xt", name=f"xt_{ip}")
                xt4 = xt.rearrange("d (two e b) -> d two e b", two=2, e=E)
                nc.sync.dma_start(xt4, xt_p[ip])

                sq = sq_pool.tile([D, 2 * E * P], bf16, tag="sq",
                                  name=f"sq_{ip}")
                sq4 = sq.rearrange("d (two e b) -> d two e b", two=2, e=E)
                nc.vector.tensor_mul(sq[:], xt[:], xt[:])

                ph = ps_h.tile([P, 2 * HQ], f32, tag="ph", name=f"ph_{ip}")
                ph3 = ph.rearrange("p (two q) -> p two q", two=2)
                for jj in range(2):
                    for e in range(E):
                        nc.tensor.matmul(
                            ph3[:, jj, 0:HP], xt4[:, jj, e], w1_3[:, e],
                            start=(e == 0), stop=(e == E - 1 and not has_b1),
                        )
                    if has_b1:
                        nc.tensor.matmul(ph3[:, jj, 0:H], ones1[:],
                                         b1_sb[:], start=False, stop=True)
                    for e in range(E):
                        nc.tensor.matmul(
                            ph3[:, jj, HP + e:HP + e + 1], sq4[:, jj, e],
                            ones_d[:], start=True, stop=True,
                        )

                hg = hg_pool.tile([P, 2 * H], bf16, tag="hg", name=f"hg_{ip}")
                act_ordered(nc.scalar.activation(
                    hg.rearrange("p (two h) -> p two h", two=2),
                    ph3[:, :, 0:H], AF.Gelu, bias=0.0, scale=1.0))
                hgs.append(hg)

                nc.vector.tensor_copy(mss4[:, j0:j0 + 2, :, :],
                                      ph3[:, :, H:HQ].rearrange(
                                          "p two (k e) -> p two k e", k=2))

                for jj in range(2):
                    s1 = sm_pool.tile([P, 6], f32, tag="s1",
                                      name=f"s1_{g}_{j0 + jj}")
                    nc.vector.bn_stats(s1[:], hg[:, jj * H:(jj + 1) * H])
                    nc.vector.bn_aggr(ln3[:, j0 + jj], s1[:])
            return st

        def emit_rest(g, st):
            mss4 = st["mss4"]
            ln3 = st["ln3"]
            zzr = st["zzr"]
            zzr3 = st["zzr3"]
            zz = st["zz"]
            zz3 = st["zz3"]
            zs = st["zs"]
            xfs = st["xfs"]
            hgs = st["hgs"]

            act_load(6)
            veps = grp_pool.tile([P, GROUP], f32, tag="veps")
            nc.vector.tensor_scalar(veps[:], ln3[:, :, 1], EPS, None, AO.add)
            lnv = grp_pool.tile([P, GROUP], f32, tag="lnv")
            act_ordered(nc.scalar.activation(lnv[:], veps[:], AF.Ln,
                                             bias=0.0, scale=1.0))
            rs1 = grp_pool.tile([P, GROUP], f32, tag="rs1")
            act_ordered(nc.scalar.activation(rs1[:], lnv[:], AF.Exp,
                                             bias=0.0, scale=-0.5))
            mb1n = grp_pool.tile([P, GROUP], f32, tag="mb1n")
            nc.vector.scalar_tensor_tensor(mb1n[:], ln3[:, :, 0], -1.0, rs1[:],
                                           AO.mult, AO.mult)

            for jp in range(NPAIR):
                j0 = 2 * jp
                pt = ps_t.tile([P, 2 * P], bf16, tag="pt", name=f"pt_{g}_{jp}")
                for jj in range(2):
                    j = j0 + jj
                    hln = sm_pool.tile([P, H], bf16, tag="hln",
                                       name=f"hln_{g}_{j}")
                    hgj = hgs[jp][:, jj * H:(jj + 1) * H]
                    if HLN_ENGINE == "s":
                        nc.scalar.activation(
                            hln[:], hgj, AF.Identity,
                            bias=mb1n[:, j:j + 1], scale=rs1[:, j:j + 1])
                    else:
                        ENG[HLN_ENGINE].tensor_scalar(
                            hln[:], hgj, rs1[:, j:j + 1], mb1n[:, j:j + 1],
                            AO.mult, AO.add)
                    if has_ln1:
                        nc.vector.tensor_mul(hln[:], hln[:], gln_sb[:])
                        nc.vector.tensor_add(hln[:], hln[:], bln_sb[:])
                    nc.tensor.matmul(pt[:, jj * P:(jj + 1) * P], hln[:],
                                     ident_b[:], is_transpose=True)
                hlt = sm_pool.tile([P, 2 * P], bf16, tag="hlt",
                                   name=f"hlt_{g}_{jp}")
                nc.scalar.activation(hlt[:], pt[:], AF.Copy)
                plg = ps_lg.tile([P, 2 * E], f32, tag="plg", name=f"plg_{g}_{jp}")
                for jj in range(2):
                    nc.tensor.matmul(plg[:, jj * E:(jj + 1) * E],
                                     hlt[:, jj * P:(jj + 1) * P], w2_sb[:],
                                     start=True, stop=True)
                nc.scalar.activation(zzr3[:, j0:j0 + 2], plg[:], AF.Copy)

            zzr3 = st["zzr3"]
            act_ordered(nc.scalar.activation(zz[:], zzr[:], AF.Exp,
                                             bias=0.0, scale=1.0))
            if has_b2:
                for j in range(GROUP):
                    nc.vector.tensor_mul(zz3[:, j], zz3[:, j], eb2_sb[:])
            nc.vector.reduce_sum(zs[:], zz3[:], axis=mybir.AxisListType.X)

            msx = mss4[:, :, 0, :]
            sqx = mss4[:, :, 1, :]
            msq = grp_pool.tile([P, GROUP * E], f32, tag="msq")
            nc.vector.scalar_tensor_tensor(msq[:], msx, 1.0 / D, msx,
                                           AO.mult, AO.mult)
            M2x = grp_pool.tile([P, GROUP * E], f32, tag="M2x")
            nc.gpsimd.tensor_sub(M2x[:], sqx, msq[:])

            ttz = grp_pool.tile([P, GROUP * E], f32, tag="ttz")
            nc.vector.tensor_mul(ttz[:], zz[:], zz[:])
            uu = grp_pool.tile([P, GROUP * E], f32, tag="uu")
            uu3 = uu.rearrange("p (j e) -> p j e", j=GROUP)
            nc.vector.tensor_mul(uu[:], ttz[:], M2x[:])
            zeps = grp_pool.tile([P, GROUP], f32, tag="zeps")
            nc.vector.scalar_tensor_tensor(zeps[:], zs[:], float(D) * EPS, zs[:],
                                           AO.mult, AO.mult)
            u2 = grp_pool.tile([P, GROUP * E], f32, tag="u2")
            u23 = u2.rearrange("p (j e) -> p j e", j=GROUP)
            for j in range(GROUP):
                nc.vector.tensor_scalar(u23[:, j], uu3[:, j], zeps[:, j:j + 1],
                                        None, AO.add)
            l2 = grp_pool.tile([P, GROUP * E], f32, tag="l2")
            act_ordered(nc.scalar.activation(l2[:], u2[:], AF.Ln,
                                             bias=0.0, scale=1.0))
            qq = grp_pool.tile([P, GROUP * E], f32, tag="qq")
            act_ordered(nc.scalar.activation(qq[:], l2[:], AF.Exp,
                                             bias=hld[:], scale=-0.5))
            aa = grp_pool.tile([P, GROUP * E], f32, tag="aa")
            aa3 = aa.rearrange("p (j e) -> p j e", j=GROUP)
            nc.vector.tensor_mul(aa[:], zz[:], qq[:])
            bbn = grp_pool.tile([P, GROUP * E], f32, tag="bbn")
            bbn3 = bbn.rearrange("p (j e) -> p j e", j=GROUP)
            nc.vector.scalar_tensor_tensor(bbn[:], msx, -1.0 / D, aa[:],
                                           AO.mult, AO.mult)

            for jp in range(NPAIR):
                j0 = 2 * jp
                ip = g * NPAIR + jp
                xf = xfs[jp]
                osb = osb_pool.tile([P, 2 * E * D], bf16, tag="osb",
                                    name=f"osb_{ip}")
                for jj in range(2):
                    j = j0 + jj
                    for e in range(E):
                        lo = (jj * E + e) * D
                        sl_o = osb[:, lo:lo + D]
                        sl_x = xf[:, lo:lo + D]
                        ename = APPLY_ENGINES[e]
                        if ename == "s":
                            nc.scalar.activation(
                                sl_o, sl_x, AF.Identity,
                                bias=bbn3[:, j, e:e + 1],
                                scale=aa3[:, j, e:e + 1])
                        else:
                            ENG[ename].tensor_scalar(
                                sl_o, sl_x,
                                aa3[:, j, e:e + 1], bbn3[:, j, e:e + 1],
                                AO.mult, AO.add,
                            )
                        if has_outgb:
                            nc.vector.tensor_mul(sl_o, sl_o, gout_sb[:])
                            nc.vector.tensor_add(sl_o, sl_o, bout_sb[:])
                nc.scalar.dma_start(
                    out_p[ip], osb.rearrange("p (two e d) -> p two e d",
                                             two=2, e=E))

        prev = None
        for g in range(n_groups):
            st = emit_phase1(g)
            if prev is not None:
                emit_rest(g - 1, prev)
            prev = st
        emit_rest(n_groups - 1, prev)

    nc.compile()
    return nc


def _get_nc(b_loc, flags, num_devices):
    key = (b_loc, flags, num_devices)
    if key not in _NC_CACHE:
        _NC_CACHE[key] = _build_nc(b_loc, *flags, num_devices=num_devices)
    return _NC_CACHE[key]


def kernel(**inputs):
    import ml_dtypes
    from concourse.bass_utils import run_bass_kernel_spmd

    features = np.asarray(inputs["features"], dtype=np.float32)
    gate_w1 = np.asarray(inputs["gate_w1"], dtype=np.float32)
    gate_b1 = np.asarray(inputs["gate_b1"], dtype=np.float32)
    ln1_g = np.asarray(inputs["ln1_g"], dtype=np.float32)
    ln1_b = np.asarray(inputs["ln1_b"], dtype=np.float32)
    gate_w2 = np.asarray(inputs["gate_w2"], dtype=np.float32)
    gate_b2 = np.asarray(inputs["gate_b2"], dtype=np.float32)
    out_g = np.asarray(inputs["out_g"], dtype=np.float32)
    out_b = np.asarray(inputs["out_b"], dtype=np.float32)

    e, B, d = features.shape
    assert e == E and d == D
    assert B % (N_CORES * P * GROUP) == 0
    b_loc = B // N_CORES
    n_tiles = b_loc // P

    has_b1 = bool(np.any(gate_b1 != 0))
    has_ln1 = bool(np.any(ln1_g != 1) or np.any(ln1_b != 0))
    has_b2 = bool(np.any(gate_b2 != 0))
    has_outgb = bool(np.any(out_g != 1) or np.any(out_b != 0))
    flags = (has_b1, has_ln1, has_b2, has_outgb)

    nc = _get_nc(b_loc, flags, num_devices=1)

    bf = ml_dtypes.bfloat16
    w1aug = np.zeros((D, E, HP), dtype=np.float32)
    w1aug[:, :, 0:H] = gate_w1.reshape(E, D, H).transpose(1, 0, 2)
    for ei in range(E):
        w1aug[:, ei, H + ei] = 1.0
    common = {
        "w1a": np.ascontiguousarray(w1aug.reshape(D, E * HP)).astype(bf),
        "w2bf": np.ascontiguousarray(gate_w2.astype(bf)),
    }
    if has_b1:
        common["b1row"] = np.ascontiguousarray(gate_b1.reshape(1, H).astype(bf))
    if has_ln1:
        common["g_ln1"] = np.ascontiguousarray(np.tile(ln1_g, (P, 1)).astype(bf))
        common["b_ln1"] = np.ascontiguousarray(np.tile(ln1_b, (P, 1)).astype(bf))
    if has_b2:
        common["eb2"] = np.ascontiguousarray(
            np.tile(np.exp(gate_b2.astype(np.float64)).astype(np.float32), (P, 1)))
    if has_outgb:
        common["g_out"] = np.ascontiguousarray(np.tile(out_g, (P, 1)).astype(bf))
        common["b_out"] = np.ascontiguousarray(np.tile(out_b, (P, 1)).astype(bf))

    fco = features.reshape(E, N_CORES, n_tiles, P, D)
    xf_all = fco.transpose(1, 2, 3, 0, 4).astype(bf)
    xt_all = fco.transpose(1, 2, 4, 0, 3).astype(bf)

    in_maps = []
    for c in range(N_CORES):
        m = dict(common)
        m["xf"] = np.ascontiguousarray(xf_all[c].reshape(b_loc, E * D))
        m["xt"] = np.ascontiguousarray(xt_all[c].reshape(n_tiles, D, E * P))
        in_maps.append(m)

    res = run_bass_kernel_spmd(nc, in_maps, core_ids=list(range(N_CORES)))
    global LAST_RESULTS
    LAST_RESULTS = res
    out = np.empty((E, B, D), dtype=np.float32)
    for c in range(N_CORES):
        oc = res.results[c]["out"].reshape(b_loc, E, D)
        out[:, c * b_loc:(c + 1) * b_loc, :] = oc.transpose(1, 0, 2)
    return out


LAST_RESULTS = None


# revision 25
# speedup vs baseline: 1.1138x; 1.0886x over previous
import numpy as np
from contextlib import ExitStack

E = 8
D = 128
H = 128
HP = H + E
HQ = HP + E
P = 128
GROUP = 16
EPS = 1e-5
HALF_LN_D = 0.5 * float(np.log(128.0))
N_CORES = 8

APPLY_ENGINES = "vvssgggg"
HLN_ENGINE = "v"

_NC_CACHE = {}


def _build_nc(b_loc, has_b1, has_ln1, has_b2, has_outgb, num_devices=1):
    import concourse.bass as bass
    import concourse.tile as tile
    from concourse import bacc, mybir, masks

    f32 = mybir.dt.float32
    bf16 = mybir.dt.bfloat16
    AO = mybir.AluOpType
    AF = mybir.ActivationFunctionType

    assert b_loc % (P * GROUP) == 0
    n_tiles = b_loc // P
    n_groups = b_loc // (P * GROUP)
    NPAIR = GROUP // 2

    nc = bacc.Bacc(
        "TRN2",
        target_bir_lowering=False,
        debug=False,
        enable_asserts=False,
        num_devices=num_devices,
    )

    xf_d = nc.dram_tensor("xf", [b_loc, E * D], bf16, kind="ExternalInput").ap()
    xt_d = nc.dram_tensor("xt", [n_tiles, D, E * P], bf16, kind="ExternalInput").ap()
    w1 = nc.dram_tensor("w1a", [D, E * HP], bf16, kind="ExternalInput").ap()
    w2 = nc.dram_tensor("w2bf", [H, E], bf16, kind="ExternalInput").ap()
    out = nc.dram_tensor("out", [b_loc, E * D], bf16, kind="ExternalOutput").ap()
    if has_b1:
        b1row = nc.dram_tensor("b1row", [1, H], bf16, kind="ExternalInput").ap()
    if has_ln1:
        g_ln1 = nc.dram_tensor("g_ln1", [P, H], bf16, kind="ExternalInput").ap()
        b_ln1 = nc.dram_tensor("b_ln1", [P, H], bf16, kind="ExternalInput").ap()
    if has_b2:
        eb2 = nc.dram_tensor("eb2", [P, E], f32, kind="ExternalInput").ap()
    if has_outgb:
        g_out = nc.dram_tensor("g_out", [P, D], bf16, kind="ExternalInput").ap()
        b_out = nc.dram_tensor("b_out", [P, D], bf16, kind="ExternalInput").ap()

    xf_p = xf_d.rearrange("(m two p) f -> m p two f", two=2, p=P)
    out_p = out.rearrange("(m two p) f -> m p two f", two=2, p=P)
    xt_p = xt_d.rearrange("(m two) d f -> m d two f", two=2)

    with tile.TileContext(nc) as tc, ExitStack() as ctx:
        _act_prev = [None]

        def act_ordered(inst):
            ins = inst.ins
            if _act_prev[0] is not None:
                tile.add_dep_helper(ins, _act_prev[0], sync=False,
                                    reason="act-table order")
            _act_prev[0] = ins
            return inst

        def act_load(set_id):
            return act_ordered(nc.scalar.add_instruction(
                mybir.InstLoadActFuncSet(
                    name=nc.get_next_instruction_name(), ins=[], outs=[],
                    act_func_set_id=set_id)))

        ENG = {"v": nc.vector, "g": nc.gpsimd}

        const_pool = ctx.enter_context(tc.tile_pool(name="const", bufs=1))
        ident_b = const_pool.tile([P, P], bf16)
        masks.make_identity(nc, ident_b[:])
        ones_d = const_pool.tile([D, 1], bf16)
        nc.vector.memset(ones_d[:], 1.0)
        w1_sb = const_pool.tile([D, E * HP], bf16)
        w1_3 = w1_sb.rearrange("d (e h) -> d e h", e=E)
        nc.sync.dma_start(w1_sb[:], w1)
        w2_sb = const_pool.tile([H, E], bf16)
        nc.sync.dma_start(w2_sb[:], w2)
        if has_b1:
            ones1 = const_pool.tile([1, P], bf16)
            nc.vector.memset(ones1[:], 1.0)
            b1_sb = const_pool.tile([1, H], bf16)
            nc.sync.dma_start(b1_sb[:], b1row)
        if has_ln1:
            gln_sb = const_pool.tile([P, H], bf16)
            nc.sync.dma_start(gln_sb[:], g_ln1)
            bln_sb = const_pool.tile([P, H], bf16)
            nc.sync.dma_start(bln_sb[:], b_ln1)
        if has_b2:
            eb2_sb = const_pool.tile([P, E], f32)
            nc.sync.dma_start(eb2_sb[:], eb2)
        if has_outgb:
            gout_sb = const_pool.tile([P, D], bf16)
            nc.sync.dma_start(gout_sb[:], g_out)
            bout_sb = const_pool.tile([P, D], bf16)
            nc.sync.dma_start(bout_sb[:], b_out)

        hld = const_pool.tile([P, 1], f32)
        nc.vector.memset(hld[:], HALF_LN_D)

        io_pool = ctx.enter_context(tc.tile_pool(name="io", bufs=NPAIR + 2))
        xt_pool = ctx.enter_context(tc.tile_pool(name="xt", bufs=6))
        sq_pool = ctx.enter_context(tc.tile_pool(name="sq", bufs=4))
        hg_pool = ctx.enter_context(tc.tile_pool(name="hg", bufs=2 * NPAIR + 1))
        osb_pool = ctx.enter_context(tc.tile_pool(name="osb", bufs=6))
        sm_pool = ctx.enter_context(tc.tile_pool(name="sm", bufs=4))
        grp_pool = ctx.enter_context(tc.tile_pool(name="grp", bufs=2))
        ps_h = ctx.enter_context(tc.tile_pool(name="ps_h", bufs=4, space="PSUM"))
        ps_t = ctx.enter_context(tc.tile_pool(name="ps_t", bufs=2, space="PSUM"))
        ps_lg = ctx.enter_context(tc.tile_pool(name="ps_lg", bufs=2, space="PSUM"))

        def emit_phase1(g):
            st = {}
            mss = grp_pool.tile([P, GROUP * 2 * E], f32, tag="mss")
            st["mss4"] = mss4 = mss.rearrange("p (j k e) -> p j k e",
                                              j=GROUP, k=2)
            ln_mv = grp_pool.tile([P, GROUP * 2], f32, tag="ln_mv")
            st["ln3"] = ln3 = ln_mv.rearrange("p (j s) -> p j s", j=GROUP)
            zzr = grp_pool.tile([P, GROUP * E], f32, tag="zzr", name="zzr")
            st["zzr"] = zzr
            st["zzr3"] = zzr.rearrange("p (j e) -> p j e", j=GROUP)
            zz = grp_pool.tile([P, GROUP * E], f32, tag="zz", name="zz")
            st["zz"] = zz
            st["zz3"] = zz.rearrange("p (j e) -> p j e", j=GROUP)
            zs = grp_pool.tile([P, GROUP], f32, tag="zs", name="zs")
            st["zs"] = zs

            st["hgs"] = hgs = []
            act_load(10)
            for jp in range(NPAIR):
                j0 = 2 * jp
                ip = g * NPAIR + jp
                xt = xt_pool.tile([D, 2 * E * P], bf16, tag="xt", name=f"xt_{ip}")
                xt4 = xt.rearrange("d (two e b) -> d two e b", two=2, e=E)
                nc.sync.dma_start(xt4, xt_p[ip])

                sq = sq_pool.tile([D, 2 * E * P], bf16, tag="sq",
                                  name=f"sq_{ip}")
                sq4 = sq.rearrange("d (two e b) -> d two e b", two=2, e=E)
                nc.vector.tensor_mul(sq[:], xt[:], xt[:])

                ph = ps_h.tile([P, 2 * HQ], f32, tag="ph", name=f"ph_{ip}")
                ph3 = ph.rearrange("p (two q) -> p two q", two=2)
                for jj in range(2):
                    for e in range(E):
                        nc.tensor.matmul(
                            ph3[:, jj, 0:HP], xt4[:, jj, e], w1_3[:, e],
                            start=(e == 0), stop=(e == E - 1 and not has_b1),
                        )
                    if has_b1:
                        nc.tensor.matmul(ph3[:, jj, 0:H], ones1[:],
                                         b1_sb[:], start=False, stop=True)
                    for e in range(E):
                        nc.tensor.matmul(
                            ph3[:, jj, HP + e:HP + e + 1], sq4[:, jj, e],
                            ones_d[:], start=True, stop=True,
                        )

                hg = hg_pool.tile([P, 2 * H], bf16, tag="hg", name=f"hg_{ip}")
                act_ordered(nc.scalar.activation(
                    hg.rearrange("p (two h) -> p two h", two=2),
                    ph3[:, :, 0:H], AF.Gelu, bias=0.0, scale=1.0))
                hgs.append(hg)

                nc.vector.tensor_copy(mss4[:, j0:j0 + 2, :, :],
                                      ph3[:, :, H:HQ].rearrange(
                                          "p two (k e) -> p two k e", k=2))

                for jj in range(2):
                    s1 = sm_pool.tile([P, 6], f32, tag="s1",
                                      name=f"s1_{g}_{j0 + jj}")
                    nc.vector.bn_stats(s1[:], hg[:, jj * H:(jj + 1) * H])
                    nc.vector.bn_aggr(ln3[:, j0 + jj], s1[:])
            return st

        def emit_rest(g, st):
            mss4 = st["mss4"]
            ln3 = st["ln3"]
            zzr = st["zzr"]
            zzr3 = st["zzr3"]
            zz = st["zz"]
            zz3 = st["zz3"]
            zs = st["zs"]
            hgs = st["hgs"]

            xfs = []
            for jp in range(NPAIR):
                ip = g * NPAIR + jp
                xf = io_pool.tile([P, 2 * E * D], bf16, tag="xf", name=f"xf_{ip}")
                xf4 = xf.rearrange("p (two e d) -> p two e d", two=2, e=E)
                nc.sync.dma_start(xf4, xf_p[ip])
                xfs.append(xf)

            act_load(6)
            veps = grp_pool.tile([P, GROUP], f32, tag="veps")
            nc.vector.tensor_scalar(veps[:], ln3[:, :, 1], EPS, None, AO.add)
            lnv = grp_pool.tile([P, GROUP], f32, tag="lnv")
            act_ordered(nc.scalar.activation(lnv[:], veps[:], AF.Ln,
                                             bias=0.0, scale=1.0))
            rs1 = grp_pool.tile([P, GROUP], f32, tag="rs1")
            act_ordered(nc.scalar.activation(rs1[:], lnv[:], AF.Exp,
                                             bias=0.0, scale=-0.5))
            mb1n = grp_pool.tile([P, GROUP], f32, tag="mb1n")
            nc.vector.scalar_tensor_tensor(mb1n[:], ln3[:, :, 0], -1.0, rs1[:],
                                           AO.mult, AO.mult)

            for jp in range(NPAIR):
                j0 = 2 * jp
                pt = ps_t.tile([P, 2 * P], bf16, tag="pt", name=f"pt_{g}_{jp}")
                for jj in range(2):
                    j = j0 + jj
                    hln = sm_pool.tile([P, H], bf16, tag="hln",
                                       name=f"hln_{g}_{j}")
                    hgj = hgs[jp][:, jj * H:(jj + 1) * H]
                    if HLN_ENGINE == "s":
                        nc.scalar.activation(
                            hln[:], hgj, AF.Identity,
                            bias=mb1n[:, j:j + 1], scale=rs1[:, j:j + 1])
                    else:
                        ENG[HLN_ENGINE].tensor_scalar(
                            hln[:], hgj, rs1[:, j:j + 1], mb1n[:, j:j + 1],
                            AO.mult, AO.add)
                    if has_ln1:
                        nc.vector.tensor_mul(hln[:], hln[:], gln_sb[:])
                        nc.vector.tensor_add(hln[:], hln[:], bln_sb[:])
                    nc.tensor.matmul(pt[:, jj * P:(jj + 1) * P], hln[:],
                                     ident_b[:], is_transpose=True)
                hlt = sm_pool.tile([P, 2 * P], bf16, tag="hlt",
                                   name=f"hlt_{g}_{jp}")
                nc.scalar.activation(hlt[:], pt[:], AF.Copy)
                plg = ps_lg.tile([P, 2 * E], f32, tag="plg", name=f"plg_{g}_{jp}")
                for jj in range(2):
                    nc.tensor.matmul(plg[:, jj * E:(jj + 1) * E],
                                     hlt[:, jj * P:(jj + 1) * P], w2_sb[:],
                                     start=True, stop=True)
                nc.scalar.activation(zzr3[:, j0:j0 + 2], plg[:], AF.Copy)

            zzr3 = st["zzr3"]
            act_ordered(nc.scalar.activation(zz[:], zzr[:], AF.Exp,
                                             bias=0.0, scale=1.0))
            if has_b2:
                for j in range(GROUP):
                    nc.vector.tensor_mul(zz3[:, j], zz3[:, j], eb2_sb[:])
            nc.vector.reduce_sum(zs[:], zz3[:], axis=mybir.AxisListType.X)

            msx = mss4[:, :, 0, :]
            sqx = mss4[:, :, 1, :]
            msq = grp_pool.tile([P, GROUP * E], f32, tag="msq")
            nc.vector.scalar_tensor_tensor(msq[:], msx, 1.0 / D, msx,
                                           AO.mult, AO.mult)
            M2x = grp_pool.tile([P, GROUP * E], f32, tag="M2x")
            nc.gpsimd.tensor_sub(M2x[:], sqx, msq[:])

            ttz = grp_pool.tile([P, GROUP * E], f32, tag="ttz")
            nc.vector.tensor_mul(ttz[:], zz[:], zz[:])
            uu = grp_pool.tile([P, GROUP * E], f32, tag="uu")
            uu3 = uu.rearrange("p (j e) -> p j e", j=GROUP)
            nc.vector.tensor_mul(uu[:], ttz[:], M2x[:])
            zeps = grp_pool.tile([P, GROUP], f32, tag="zeps")
            nc.vector.scalar_tensor_tensor(zeps[:], zs[:], float(D) * EPS, zs[:],
                                           AO.mult, AO.mult)
            u2 = grp_pool.tile([P, GROUP * E], f32, tag="u2")
            u23 = u2.rearrange("p (j e) -> p j e", j=GROUP)
            for j in range(GROUP):
                nc.vector.tensor_scalar(u23[:, j], uu3[:, j], zeps[:, j:j + 1],
                                        None, AO.add)
            l2 = grp_pool.tile([P, GROUP * E], f32, tag="l2")
            act_ordered(nc.scalar.activation(l2[:], u2[:], AF.Ln,
                                             bias=0.0, scale=1.0))
            qq = grp_pool.tile([P, GROUP * E], f32, tag="qq")
            act_ordered(nc.scalar.activation(qq[:], l2[:], AF.Exp,
                                             bias=hld[:], scale=-0.5))
            aa = grp_pool.tile([P, GROUP * E], f32, tag="aa")
            aa3 = aa.rearrange("p (j e) -> p j e", j=GROUP)
            nc.vector.tensor_mul(aa[:], zz[:], qq[:])
            bbn = grp_pool.tile([P, GROUP * E], f32, tag="bbn")
            bbn3 = bbn.rearrange("p (j e) -> p j e", j=GROUP)
            nc.vector.scalar_tensor_tensor(bbn[:], msx, -1.0 / D, aa[:],
                                           AO.mult, AO.mult)

            for jp in range(NPAIR):
                j0 = 2 * jp
                ip = g * NPAIR + jp
                xf = xfs[jp]
                osb = osb_pool.tile([P, 2 * E * D], bf16, tag="osb",
                                    name=f"osb_{ip}")
                for jj in range(2):
                    j = j0 + jj
                    for e in range(E):
                        lo = (jj * E + e) * D
                        sl_o = osb[:, lo:lo + D]
                        sl_x = xf[:, lo:lo + D]
                        ename = APPLY_ENGINES[e]
                        if ename == "s":
                            nc.scalar.activation(
                                sl_o, sl_x, AF.Identity,
                                bias=bbn3[:, j, e:e + 1],
                                scale=aa3[:, j, e:e + 1])
                        else:
                            ENG[ename].tensor_scalar(
                                sl_o, sl_x,
                                aa3[:, j, e:e + 1], bbn3[:, j, e:e + 1],
                                AO.mult, AO.add,
                            )
                        if has_outgb:
                            nc.vector.tensor_mul(sl_o, sl_o, gout_sb[:])
                            nc.vector.tensor_add(sl_o, sl_o, bout_sb[:])
                nc.scalar.dma_start(
                    out_p[ip], osb.rearrange("p (two e d) -> p two e d",
                                             two=2, e=E))

        prev = None
        for g in range(n_groups):
            st = emit_phase1(g)
            if prev is not None:
                emit_rest(g - 1, prev)
            prev = st
        emit_rest(n_groups - 1, prev)

    nc.compile()
    return nc


def _get_nc(b_loc, flags, num_devices):
    key = (b_loc, flags, num_devices)
    if key not in _NC_CACHE:
        _NC_CACHE[key] = _build_nc(b_loc, *flags, num_devices=num_devices)
    return _NC_CACHE[key]


def kernel(**inputs):
    import ml_dtypes
    from concourse.bass_utils import run_bass_kernel_spmd

    features = np.asarray(inputs["features"], dtype=np.float32)
    gate_w1 = np.asarray(inputs["gate_w1"], dtype=np.float32)
    gate_b1 = np.asarray(inputs["gate_b1"], dtype=np.float32)
    ln1_g = np.asarray(inputs["ln1_g"], dtype=np.float32)
    ln1_b = np.asarray(inputs["ln1_b"], dtype=np.float32)
    gate_w2 = np.asarray(inputs["gate_w2"], dtype=np.float32)
    gate_b2 = np.asarray(inputs["gate_b2"], dtype=np.float32)
    out_g = np.asarray(inputs["out_g"], dtype=np.float32)
    out_b = np.asarray(inputs["out_b"], dtype=np.float32)

    e, B, d = features.shape
    assert e == E and d == D
    assert B % (N_CORES * P * GROUP) == 0
    b_loc = B // N_CORES
    n_tiles = b_loc // P

    has_b1 = bool(np.any(gate_b1 != 0))
    has_ln1 = bool(np.any(ln1_g != 1) or np.any(ln1_b != 0))
    has_b2 = bool(np.any(gate_b2 != 0))
    has_outgb = bool(np.any(out_g != 1) or np.any(out_b != 0))
    flags = (has_b1, has_ln1, has_b2, has_outgb)

    nc = _get_nc(b_loc, flags, num_devices=1)

    bf = ml_dtypes.bfloat16
    w1aug = np.zeros((D, E, HP), dtype=np.float32)
    w1aug[:, :, 0:H] = gate_w1.reshape(E, D, H).transpose(1, 0, 2)
    for ei in range(E):
        w1aug[:, ei, H + ei] = 1.0
    common = {
        "w1a": np.ascontiguousarray(w1aug.reshape(D, E * HP)).astype(bf),
        "w2bf": np.ascontiguousarray(gate_w2.astype(bf)),
    }
    if has_b1:
        common["b1row"] = np.ascontiguousarray(gate_b1.reshape(1, H).astype(bf))
    if has_ln1:
        common["g_ln1"] = np.ascontiguousarray(np.tile(ln1_g, (P, 1)).astype(bf))
        common["b_ln1"] = np.ascontiguousarray(np.tile(ln1_b, (P, 1)).astype(bf))
    if has_b2:
        common["eb2"] = np.ascontiguousarray(
            np.tile(np.exp(gate_b2.astype(np.float64)).astype(np.float32), (P, 1)))
    if has_outgb:
        common["g_out"] = np.ascontiguousarray(np.tile(out_g, (P, 1)).astype(bf))
        common["b_out"] = np.ascontiguousarray(np.tile(out_b, (P, 1)).astype(bf))

    fco = features.reshape(E, N_CORES, n_tiles, P, D)
    xf_all = fco.transpose(1, 2, 3, 0, 4).astype(bf)
    xt_all = fco.transpose(1, 2, 4, 0, 3).astype(bf)

    in_maps = []
    for c in range(N_CORES):
        m = dict(common)
        m["xf"] = np.ascontiguousarray(xf_all[c].reshape(b_loc, E * D))
        m["xt"] = np.ascontiguousarray(xt_all[c].reshape(n_tiles, D, E * P))
        in_maps.append(m)

    res = run_bass_kernel_spmd(nc, in_maps, core_ids=list(range(N_CORES)))
    global LAST_RESULTS
    LAST_RESULTS = res
    out = np.empty((E, B, D), dtype=np.float32)
    for c in range(N_CORES):
        oc = res.results[c]["out"].reshape(b_loc, E, D)
        out[:, c * b_loc:(c + 1) * b_loc, :] = oc.transpose(1, 0, 2)
    return out


LAST_RESULTS = None
